# revision 17
# baseline (speedup 1.0000x reference)
"""Mean-shift filtering kernel for Trainium2, SPMD over 8 NeuronCores.

Algorithm (per core): flash-attention-style streaming over the N x N
Gaussian kernel matrix. Each core owns Q = N/4 query pixels of one batch
image (cores 0-3 -> batch 0, cores 4-7 -> batch 1) and the full point set
of that image.

Math: w[m,n] = exp(-||y_n - x_m||^2 / (2 bw^2))
            = exp(100 * (y_n.x_m - 0.5||y_n||^2 - 0.5||x_m||^2))
The inner term is ONE K=15 bf16 matmul via a compensated hi/lo split:
  out1 = hiX.hiY + hiX.loY + loX.hiY   (error ~1e-5 -> exp factor err ~1e-3)
with lhsT rows [hiX5; hiX5; loX5] and rhs rows [hiY5; loY5; hiY5], where
  X5 = [x0; x1; x2; 1; -0.5||x||^2],  Y5 = [y0; y1; y2; -0.5||y||^2; 1].
Then w = Exp(100 * out1) on ScalarE (PSUM -> SBUF bf16, grouped 3 PSUM banks
per activation to amortize the per-call ACT overhead), and a second bf16
matmul accumulates [den; num] over all point chunks:
  out2[4, n] += pts2[128, 4]^T @ w[128, n]  (pts2 rows = [1, x0, x1, x2]).
Epilogue (per n-tile): r = 1/den (DVE reciprocal), broadcast across
partitions with a ones [1,4] bf16 hi/lo matmul pair, T = out2 * r = [1, y],
S = T^2, and four accumulating K=4 bf16 matmuls (hi/lo of T and S against
[4,5] coefficient blocks) rebuild [y; -0.5||y||^2; 1] in PSUM; DVE splits
that into bf16 hi/lo and three SBUF->SBUF DMAs place the Y15 row blocks for
the next iteration. All epilogue matmuls are compensated bf16 (1 cyc/col).

Scheduling: MM2 groups are emitted one group behind MM1s, and each tile's
final MM2 group + epilogue are deferred until after the next tile's second
group, so in PE program order the next tile's MM1s always precede
activation-blocked work -- keeps ScalarE gap-free across tile boundaries
(timeline-sim ACT occupancy ~90%).

PSUM: out1 2x[128,1536] (6 banks) + a shared 2-buf pool for out2/bc/yps
(2 banks) = 8 banks exactly. Engines: PE ~2 cycles/kernel-element (both
matmuls), ACT 1 element/cycle/lane (the bound), DVE/SP only epilogue work.

Cost-model timeline: 876us for the 5-iteration mean shift (ACT busy 792us);
HW wall-derived estimate ~200us/iteration, ~1.0ms total.
"""

import numpy as np
import ml_dtypes

import concourse.bass as bass
import concourse.tile as tile
from concourse import bacc, mybir
from concourse.bass_utils import run_bass_kernel_spmd

F32 = mybir.dt.float32
BF16 = mybir.dt.bfloat16

B, C, H, W = 2, 3, 96, 96
N = H * W            # 9216 points per image
NCORES = 8
CORES_PER_B = NCORES // B   # 4
Q = N // CORES_PER_B        # 2304 queries per core
NUM_ITERS = 5
BANDWIDTH = 0.1
SCALE = 1.0 / (BANDWIDTH * BANDWIDTH)  # 100.0 ; exp arg = SCALE * out1
CHUNK = 128
NCHUNK = N // CHUNK  # 72
# n-tiles within a core's Q queries: 512-wide (PSUM-bank aligned) + tail.
NTILES = [(0, 512), (512, 512), (1024, 512), (1536, 512), (2048, 256)]
GROUPW = 1536        # ACT group width = 3 PSUM banks of fp32


def _emit(nc, tc, aps, num_iters=NUM_ITERS, groupw=GROUPW, o1bufs=2):
    paug, pts2, y015, ones18, amat, yout = (
        aps["paug"], aps["pts2"], aps["y015"], aps["ones18"], aps["amat"],
        aps["yout"])

    import contextlib
    ctx = contextlib.ExitStack()
    cpool = ctx.enter_context(tc.tile_pool(name="const", bufs=1))
    ypool = ctx.enter_context(tc.tile_pool(name="ybuf", bufs=2))
    wpool = ctx.enter_context(tc.tile_pool(name="w", bufs=3))
    spool = ctx.enter_context(tc.tile_pool(name="small", bufs=2))
    o1pool = ctx.enter_context(tc.tile_pool(name="out1", bufs=o1bufs, space="PSUM"))
    o2pool = ctx.enter_context(tc.tile_pool(name="out2", bufs=2, space="PSUM"))

    # resident inputs; paug DRAM is [10, N] = [hiX5; loX5], SBUF wants
    # [hiX5; hiX5; loX5] (pairs with Y15 = [hiY5; loY5; hiY5])
    # Load order: the first tile's dependencies (ya, paug) first so the
    # pipeline starts as early as possible; pts2/amat arrive under compute.
    ya = ypool.tile([15, Q], BF16, tag="ybuf")
    yb = ypool.tile([15, Q], BF16, tag="ybuf")
    nc.sync.dma_start(ya[0:5, :], y015[0:5, :])
    nc.sync.dma_start(ya[5:10, :], y015[5:10, :])
    nc.sync.dma_start(ya[10:15, :], y015[0:5, :])
    paug_t = cpool.tile([15, N], BF16, tag="paug")
    nc.sync.dma_start(paug_t[0:5, :], paug[0:5, :])
    nc.sync.dma_start(paug_t[5:10, :], paug[0:5, :])
    nc.sync.dma_start(paug_t[10:15, :], paug[5:10, :])
    pts2_t = cpool.tile([128, 4 * NCHUNK], BF16, tag="pts2")
    nc.sync.dma_start(pts2_t[:], pts2[:])
    ones18_t = cpool.tile([1, 4], BF16, tag="ones18")
    nc.sync.dma_start(ones18_t[:], ones18[:])
    amat_t = cpool.tile([4, 20], BF16, tag="amat")
    nc.sync.dma_start(amat_t[:], amat[:])
    yout_t = cpool.tile([3, Q], F32, tag="youtb")

    exp_fn = mybir.ActivationFunctionType.Exp

    pending = []
    for t in range(num_iters):
        ycur = ya if t % 2 == 0 else yb
        ynext = yb if t % 2 == 0 else ya
        last = t == num_iters - 1
        for (off, nT) in NTILES:
            gsz = groupw // nT  # chunks per ACT group
            out2 = o2pool.tile([4, nT], F32, tag="out2")

            def mm2(g, w, gsz=gsz, nT=nT, out2=out2):
                for j in range(gsz):
                    ch = g * gsz + j
                    nc.tensor.matmul(
                        out2[:],
                        pts2_t[:, ch * 4:(ch + 1) * 4],
                        w[:, j * nT:(j + 1) * nT],
                        start=(ch == 0), stop=(ch == NCHUNK - 1))

            # MM2s are emitted one group behind MM1s so that, in PE program
            # order, the next group's (and next tile's) MM1s precede MM2s
            # that block on the activation -- keeps ACT gap-free across
            # tile boundaries.
            mm2_prev = None
            for g in range(NCHUNK // gsz):
                out1 = o1pool.tile([128, groupw], F32, tag="out1")
                for j in range(gsz):
                    ch = g * gsz + j
                    nc.tensor.matmul(
                        out1[:, j * nT:(j + 1) * nT],
                        paug_t[:, ch * CHUNK:(ch + 1) * CHUNK],
                        ycur[:, off:off + nT],
                        start=True, stop=True)
                w = wpool.tile([128, groupw], BF16, tag="w")
                nc.scalar.activation(w[:], out1[:], exp_fn, scale=SCALE)
                if mm2_prev is not None:
                    mm2(*mm2_prev)
                mm2_prev = (g, w)
                if g == 1 and pending:
                    pending.pop(0)()
            mm2_last = mm2_prev
            # epilogue: divide, rebuild Y15 (or final output). All epilogue
            # matmuls are compensated hi/lo bf16 (1 cyc/col on PE instead of
            # fp32's 4). Emission is deferred (see below) so the scheduler
            # keeps feeding ACT with the next tile's groups first.
            def epilogue(out2=out2, off=off, nT=nT, last=last, ynext=ynext,
                         mm2=mm2, mm2_last=mm2_last):
                mm2(*mm2_last)  # deferred final MM2 group of this tile
                r = spool.tile([1, nT], F32, tag="r")
                nc.vector.reciprocal(r[:], out2[0:1, :])
                # free out2's PSUM slot early (two out2 tiles are live under
                # deferred emission; bc below needs a slot from the same pool)
                o2c = spool.tile([4, nT], F32, tag="o2c")
                nc.vector.tensor_copy(o2c[:], out2[:])
                # hi/lo split of r; broadcast via two accumulating bf16 mms
                rhi = spool.tile([1, nT], BF16, tag="rhi")
                nc.vector.tensor_copy(rhi[:], r[:])
                rlo = spool.tile([1, nT], BF16, tag="rlo")
                nc.vector.tensor_sub(rlo[:], r[:], rhi[:])
                bc = o2pool.tile([4, nT], F32, tag="out2")
                nc.tensor.matmul(bc[:], ones18_t[:], rhi[:], start=True,
                                 stop=False)
                nc.tensor.matmul(bc[:], ones18_t[:], rlo[:], start=False,
                                 stop=True)
                bcS = spool.tile([4, nT], F32, tag="bcS")
                nc.vector.tensor_copy(bcS[:], bc[:])
                # T rows = [1, y0, y1, y2]; S = T^2; hi/lo of each, then
                # yps = sum of four accumulating K=4 bf16 matmuls
                T = spool.tile([4, nT], F32, tag="T")
                nc.vector.tensor_mul(T[:], o2c[:], bcS[:])
                S = spool.tile([4, nT], F32, tag="S")
                nc.vector.tensor_mul(S[:], T[:], T[:])
                thi = spool.tile([4, nT], BF16, tag="thi")
                nc.vector.tensor_copy(thi[:], T[:])
                tlo = spool.tile([4, nT], BF16, tag="tlo")
                nc.vector.tensor_sub(tlo[:], T[:], thi[:])
                shi = spool.tile([4, nT], BF16, tag="shi")
                nc.vector.tensor_copy(shi[:], S[:])
                slo = spool.tile([4, nT], BF16, tag="slo")
                nc.vector.tensor_sub(slo[:], S[:], shi[:])
                yps = o2pool.tile([5, nT], F32, tag="out2")
                for blk, rh in enumerate((thi, tlo, shi, slo)):
                    nc.tensor.matmul(yps[:], amat_t[:, 5 * blk:5 * blk + 5],
                                     rh[:], start=(blk == 0), stop=(blk == 3))
                if last:
                    nc.vector.tensor_copy(yout_t[:, off:off + nT], yps[0:3, :])
                else:
                    sl = slice(off, off + nT)
                    yhi = spool.tile([5, nT], BF16, tag="yhi")
                    nc.vector.tensor_copy(yhi[:], yps[:])
                    ylo = spool.tile([5, nT], BF16, tag="ylo")
                    nc.vector.tensor_sub(ylo[:], yps[:], yhi[:])
                    # place the three row-blocks of Y15 via SBUF->SBUF DMA
                    nc.sync.dma_start(ynext[0:5, sl], yhi[:])
                    nc.sync.dma_start(ynext[5:10, sl], ylo[:])
                    nc.sync.dma_start(ynext[10:15, sl], yhi[:])
            pending.append(epilogue)

    while pending:
        pending.pop(0)()
    nc.sync.dma_start(yout[:], yout_t[:])
    ctx.close()


def build(num_iters=NUM_ITERS, groupw=GROUPW, o1bufs=2):
    nc = bacc.Bacc("TRN2", target_bir_lowering=False, debug=False)
    aps = {
        "paug": nc.dram_tensor("paug", [10, N], BF16, kind="ExternalInput").ap(),
        "pts2": nc.dram_tensor("pts2", [128, 4 * NCHUNK], BF16,
                               kind="ExternalInput").ap(),
        "y015": nc.dram_tensor("y015", [10, Q], BF16, kind="ExternalInput").ap(),
        "ones18": nc.dram_tensor("ones18", [1, 4], BF16, kind="ExternalInput").ap(),
        "amat": nc.dram_tensor("amat", [4, 20], BF16, kind="ExternalInput").ap(),
        "yout": nc.dram_tensor("yout", [3, Q], F32, kind="ExternalOutput").ap(),
    }
    with tile.TileContext(nc) as tc:
        _emit(nc, tc, aps, num_iters, groupw, o1bufs)
    nc.compile()
    return nc


def _hi_lo(a):
    """Split fp32 array into bf16 hi + bf16 lo (a ~ hi + lo)."""
    hi = a.astype(ml_dtypes.bfloat16)
    lo = (a - hi.astype(np.float32)).astype(ml_dtypes.bfloat16)
    return hi, lo


def _x5(p):
    """[5, n] rows [x0;x1;x2;1;-0.5||x||^2] for points p [n, 3] (lhsT side)."""
    n = p.shape[0]
    return np.concatenate(
        [p.T, np.ones((1, n), np.float32),
         -0.5 * (p * p).sum(1, dtype=np.float32)[None, :]], 0)


def _y5(p):
    """[5, n] rows [y0;y1;y2;-0.5||y||^2;1] for queries p [n, 3] (rhs side)."""
    n = p.shape[0]
    return np.concatenate(
        [p.T, -0.5 * (p * p).sum(1, dtype=np.float32)[None, :],
         np.ones((1, n), np.float32)], 0)


def make_in_maps(x):
    x = np.asarray(x, dtype=np.float32)
    ones18 = np.ones((1, 4), ml_dtypes.bfloat16)
    # amat [4, 20]: four [4, 5] column blocks applied to T_hi, T_lo, S_hi,
    # S_lo; output rows = [y0, y1, y2, -0.5||y||^2, 1]
    amat = np.zeros((4, 20), np.float32)
    for blk in (0, 5):          # T_hi / T_lo blocks
        for j in range(3):
            amat[1 + j, blk + j] = 1.0   # y_j
        amat[0, blk + 4] = 1.0           # ones row
    for blk in (10, 15):        # S_hi / S_lo blocks
        for j in range(3):
            amat[1 + j, blk + 3] = -0.5  # -0.5*sum y_j^2
    amat = amat.astype(ml_dtypes.bfloat16)
    in_maps = []
    for c in range(NCORES):
        b = c // CORES_PER_B
        pts = x[b].reshape(C, N).T.copy()          # [N, 3]
        q = pts[(c % CORES_PER_B) * Q:(c % CORES_PER_B + 1) * Q]  # [Q, 3]
        hiX, loX = _hi_lo(_x5(pts))
        paug = np.concatenate([hiX, loX], 0)       # [10, N] bf16
        hiY, loY = _hi_lo(_y5(q))
        y015 = np.concatenate([hiY, loY], 0)       # [10, Q] bf16
        a = pts.reshape(NCHUNK, CHUNK, C)
        cols = np.concatenate(
            [np.ones((NCHUNK, CHUNK, 1), np.float32), a], -1)  # [72,128,4]
        pts2 = np.ascontiguousarray(
            cols.transpose(1, 0, 2).reshape(CHUNK, 4 * NCHUNK)
        ).astype(ml_dtypes.bfloat16)
        in_maps.append({
            "paug": paug, "pts2": pts2, "y015": y015,
            "ones18": ones18, "amat": amat,
        })
    return in_maps


def assemble(results):
    y = np.empty((B, C, N), np.float32)
    for c in range(NCORES):
        b = c // CORES_PER_B
        sl = slice((c % CORES_PER_B) * Q, (c % CORES_PER_B + 1) * Q)
        y[b, :, sl] = results[c]["yout"]
    return y.reshape(B, C, H, W)


class _CachedRunner:
    """run_bass_kernel_spmd's axon path (bass2jax.run_bass_via_pjrt) with the
    jitted SPMD executable cached across calls, so repeat invocations skip
    re-tracing/lowering. Math and execution mechanism are identical."""

    def __init__(self, nc, n_cores=NCORES):
        import jax
        from jax.sharding import Mesh, PartitionSpec
        from jax.experimental.shard_map import shard_map
        from concourse import bass2jax
        import concourse.mybir as mybir_

        bass2jax.install_neuronx_cc_hook()
        self.jax = jax
        in_names, out_names, out_avals, zero_outs = [], [], [], []
        partition_name = (nc.partition_id_tensor.name
                          if nc.partition_id_tensor else None)
        for alloc in nc.m.functions[0].allocations:
            if not isinstance(alloc, mybir_.MemoryLocationSet):
                continue
            name = alloc.memorylocations[0].name
            if alloc.kind == "ExternalInput":
                if name != partition_name:
                    in_names.append(name)
            elif alloc.kind == "ExternalOutput":
                out_names.append(name)
                shape = tuple(alloc.tensor_shape)
                dtype = mybir_.dt.np(alloc.dtype)
                out_avals.append(jax.core.ShapedArray(shape, dtype))
                zero_outs.append(np.zeros(shape, dtype))
        self.n_cores = n_cores
        self.in_names, self.out_names = in_names, out_names
        self.out_avals = out_avals
        self.zeros = [np.zeros((n_cores * z.shape[0], *z.shape[1:]), z.dtype)
                      for z in zero_outs]
        n_params, n_outs = len(in_names), len(out_avals)
        all_in = in_names + out_names
        if partition_name is not None:
            all_in = all_in + [partition_name]

        def _body(*args):
            operands = list(args)
            if partition_name is not None:
                operands.append(bass2jax.partition_id_tensor())
            return tuple(bass2jax._bass_exec_p.bind(
                *operands,
                out_avals=tuple(out_avals),
                in_names=tuple(all_in),
                out_names=tuple(out_names),
                lowering_input_output_aliases=(),
                sim_require_finite=True,
                sim_require_nnan=True,
                nc=nc,
            ))

        devices = jax.devices()[:n_cores]
        mesh = Mesh(np.asarray(devices), ("core",))
        self.fn = jax.jit(
            shard_map(_body, mesh=mesh,
                      in_specs=(PartitionSpec("core"),) * (n_params + n_outs),
                      out_specs=(PartitionSpec("core"),) * n_outs,
                      check_rep=False),
            donate_argnums=tuple(range(n_params, n_params + n_outs)),
            keep_unused=True,
        )

    def __call__(self, in_maps):
        per_core = [[np.asarray(m[n]) for n in self.in_names] for m in in_maps]
        concat_in = [
            np.concatenate([per_core[c][i] for c in range(self.n_cores)], 0)
            for i in range(len(self.in_names))]
        out = self.fn(*concat_in, *self.zeros)
        pulled = [np.asarray(o).reshape(self.n_cores, *av.shape)
                  for o, av in zip(out, self.out_avals)]
        return [{n: pulled[i][c] for i, n in enumerate(self.out_names)}
                for c in range(self.n_cores)]


_NC = None
_RUNNER = None


def kernel(x):
    global _NC, _RUNNER
    if _NC is None:
        _NC = build()
    in_maps = make_in_maps(x)
    if _RUNNER is None:
        try:
            _RUNNER = _CachedRunner(_NC)
        except Exception:
            _RUNNER = False
    if _RUNNER:
        try:
            return assemble(_RUNNER(in_maps))
        except Exception:
            pass
    res = run_bass_kernel_spmd(_NC, in_maps, core_ids=list(range(NCORES)))
    return assemble(res.results)


# revision 23
# speedup vs baseline: 1.7555x; 1.7555x over previous
"""Mean-shift filtering kernel for Trainium2, SPMD over 8 NeuronCores.

Algorithm (per core): flash-attention-style streaming over the N x N
Gaussian kernel matrix. Each core owns Q = N/4 query pixels of one batch
image (cores 0-3 -> batch 0, cores 4-7 -> batch 1) and the full point set
of that image.

Math: w[m,n] = exp(-||y_n - x_m||^2 / (2 bw^2))
            = exp(100 * (y_n.x_m - 0.5||y_n||^2 - 0.5||x_m||^2))
The inner term is ONE K=15 bf16 matmul via a compensated hi/lo split:
  out1 = hiX.hiY + hiX.loY + loX.hiY   (error ~1e-5 -> exp factor err ~1e-3)
with lhsT rows [hiX5; hiX5; loX5] and rhs rows [hiY5; loY5; hiY5], where
  X5 = [x0; x1; x2; 1; -0.5||x||^2],  Y5 = [y0; y1; y2; -0.5||y||^2; 1].
Then w = Exp(100 * out1) on ScalarE (PSUM -> SBUF bf16, grouped 3 PSUM banks
per activation to amortize the per-call ACT overhead), and a second bf16
matmul accumulates [den; num] over all point chunks:
  out2[4, n] += pts2[128, 4]^T @ w[128, n]  (pts2 rows = [1, x0, x1, x2]).
Epilogue (per n-tile): PE-free. r = 1/den (DVE reciprocal); the partition
broadcast of r, the cross-partition sum for ||y||^2, and all Y15 row
placements are single-row SBUF->SBUF DMAs (DMA is the only partition-crossing
engine; thin-K matmuls measured ~3-4x their streaming cost on HW, so PE is
kept exclusively for the two dense streams). DVE does the multiplies and the
bf16 hi/lo splits.

Scheduling: MM2 groups are emitted one group behind MM1s, and each tile's
final MM2 group + epilogue are deferred until after the next tile's second
group, so in PE program order the next tile's MM1s always precede
activation-blocked work -- keeps ScalarE gap-free across tile boundaries
(timeline-sim ACT occupancy ~90%).

MM1's stationary operand and rhs are K-padded from 15 to 128 rows with
zeros: matmul streaming cost is K-independent, and thin-K moving operands
measured ~600ns/matmul slower on HW (plus 128-row weights enable the fast
weight load path) -- worth ~215us/iteration.

PSUM: out1 2x[128,1536] (6 banks) + out2 2x[4,512] (2 banks) = 8 banks.
Engines: PE 2 cycles/kernel-element (the two dense matmul streams), ACT 1
element/cycle/lane, DVE + DMA queues take the whole epilogue.

HW wall-derived estimate ~135-180us/iteration, ~0.7-0.9ms for the full
5-iteration mean shift (pure-exp ScalarE roofline: 138us/iteration).
"""

import numpy as np
import ml_dtypes

import concourse.bass as bass
import concourse.tile as tile
from concourse import bacc, mybir
from concourse.bass_utils import run_bass_kernel_spmd

F32 = mybir.dt.float32
BF16 = mybir.dt.bfloat16

B, C, H, W = 2, 3, 96, 96
N = H * W            # 9216 points per image
NCORES = 8
CORES_PER_B = NCORES // B   # 4
Q = N // CORES_PER_B        # 2304 queries per core
NUM_ITERS = 5
BANDWIDTH = 0.1
SCALE = 1.0 / (BANDWIDTH * BANDWIDTH)  # 100.0 ; exp arg = SCALE * out1
CHUNK = 128
NCHUNK = N // CHUNK  # 72
# n-tiles within a core's Q queries: 512-wide (PSUM-bank aligned) + tail.
NTILES = [(0, 512), (512, 512), (1024, 512), (1536, 512), (2048, 256)]
GROUPW = 1536        # ACT group width = 3 PSUM banks of fp32


def _emit(nc, tc, aps, num_iters=NUM_ITERS, groupw=GROUPW, o1bufs=2, ntiles=None):
    paug, pts2, y015, yout = (
        aps["paug"], aps["pts2"], aps["y015"], aps["yout"])

    import contextlib
    ctx = contextlib.ExitStack()
    cpool = ctx.enter_context(tc.tile_pool(name="const", bufs=1))
    ypool = ctx.enter_context(tc.tile_pool(name="ybuf", bufs=2))
    wpool = ctx.enter_context(tc.tile_pool(name="w", bufs=3))
    spool = ctx.enter_context(tc.tile_pool(name="small", bufs=2))
    o1pool = ctx.enter_context(tc.tile_pool(name="out1", bufs=o1bufs, space="PSUM"))
    o2pool = ctx.enter_context(tc.tile_pool(name="out2", bufs=2, space="PSUM"))

    # resident inputs; paug DRAM is [10, N] = [hiX5; loX5], SBUF wants
    # [hiX5; hiX5; loX5] (pairs with Y15 = [hiY5; loY5; hiY5])
    # Load order: the first tile's dependencies (ya, paug) first so the
    # pipeline starts as early as possible; pts2/amat arrive under compute.
    ya = ypool.tile([128, Q], BF16, tag="ybuf")
    yb = ypool.tile([128, Q], BF16, tag="ybuf")
    nc.vector.memset(ya[:], 0.0)
    nc.vector.memset(yb[:], 0.0)
    nc.sync.dma_start(ya[0:5, :], y015[0:5, :])
    nc.sync.dma_start(ya[5:10, :], y015[5:10, :])
    nc.sync.dma_start(ya[10:15, :], y015[0:5, :])
    # yb's constant rows (ones row hi=1 / lo=0 and its duplicate)
    nc.sync.dma_start(yb[4:5, :], y015[4:5, :])
    nc.sync.dma_start(yb[9:10, :], y015[9:10, :])
    nc.sync.dma_start(yb[14:15, :], y015[4:5, :])
    # K padded 15 -> 128 with zero rows: streaming cost is K-independent and
    # 128-column/128-row weights enable the fast-weight-load path.
    paug_t = cpool.tile([128, N], BF16, tag="paug")
    nc.vector.memset(paug_t[:], 0.0)
    nc.sync.dma_start(paug_t[0:5, :], paug[0:5, :])
    nc.sync.dma_start(paug_t[5:10, :], paug[0:5, :])
    nc.sync.dma_start(paug_t[10:15, :], paug[5:10, :])
    pts2_t = cpool.tile([128, 4 * NCHUNK], BF16, tag="pts2")
    nc.sync.dma_start(pts2_t[:], pts2[:])
    yout_t = cpool.tile([3, Q], F32, tag="youtb")

    exp_fn = mybir.ActivationFunctionType.Exp

    pending = []
    for t in range(num_iters):
        ycur = ya if t % 2 == 0 else yb
        ynext = yb if t % 2 == 0 else ya
        last = t == num_iters - 1
        for (off, nT) in (ntiles or NTILES):
            gsz = groupw // nT  # chunks per ACT group
            out2 = o2pool.tile([4, nT], F32, tag="out2")

            def mm2(g, w, gsz=gsz, nT=nT, out2=out2):
                for j in range(gsz):
                    ch = g * gsz + j
                    nc.tensor.matmul(
                        out2[:],
                        pts2_t[:, ch * 4:(ch + 1) * 4],
                        w[:, j * nT:(j + 1) * nT],
                        start=(ch == 0), stop=(ch == NCHUNK - 1))

            # MM2s are emitted one group behind MM1s so that, in PE program
            # order, the next group's (and next tile's) MM1s precede MM2s
            # that block on the activation -- keeps ACT gap-free across
            # tile boundaries.
            mm2_prev = None
            for g in range(NCHUNK // gsz):
                out1 = o1pool.tile([128, groupw], F32, tag="out1")
                for j in range(gsz):
                    ch = g * gsz + j
                    nc.tensor.matmul(
                        out1[:, j * nT:(j + 1) * nT],
                        paug_t[:, ch * CHUNK:(ch + 1) * CHUNK],
                        ycur[:, off:off + nT],
                        start=True, stop=True)
                w = wpool.tile([128, groupw], BF16, tag="w")
                nc.scalar.activation(w[:], out1[:], exp_fn, scale=SCALE)
                if mm2_prev is not None:
                    mm2(*mm2_prev)
                mm2_prev = (g, w)
                if g == 1 and pending:
                    pending.pop(0)()
            mm2_last = mm2_prev
            # epilogue: divide, rebuild Y15 (or final output). All epilogue
            # matmuls are compensated hi/lo bf16 (1 cyc/col on PE instead of
            # fp32's 4). Emission is deferred (see below) so the scheduler
            # keeps feeding ACT with the next tile's groups first.
            def epilogue(out2=out2, off=off, nT=nT, last=last, ynext=ynext,
                         mm2=mm2, mm2_last=mm2_last):
                mm2(*mm2_last)  # deferred final MM2 group of this tile
                # PE-free epilogue: broadcasts and partition moves via SBUF
                # DMAs, arithmetic on DVE only (thin-K matmuls are slow on HW)
                r = spool.tile([1, nT], F32, tag="r")
                nc.vector.reciprocal(r[:], out2[0:1, :])
                o2c = spool.tile([4, nT], F32, tag="o2c")
                nc.vector.tensor_copy(o2c[:], out2[:])
                bcS = spool.tile([4, nT], F32, tag="bcS")
                for k in range(4):
                    nc.sync.dma_start(bcS[k:k + 1, :], r[:])
                T = spool.tile([4, nT], F32, tag="T")
                nc.vector.tensor_mul(T[:], o2c[:], bcS[:])  # [1, y0, y1, y2]
                if last:
                    nc.sync.dma_start(yout_t[:, off:off + nT], T[1:4, :])
                    return
                S = spool.tile([4, nT], F32, tag="S")
                nc.vector.tensor_mul(S[:], T[:], T[:])
                # ysq = sum of S rows 1..3 via row DMAs to partition 0
                qa = spool.tile([1, nT], F32, tag="qa")
                nc.sync.dma_start(qa[:], S[1:2, :])
                qb = spool.tile([1, nT], F32, tag="qb")
                nc.sync.dma_start(qb[:], S[2:3, :])
                qc = spool.tile([1, nT], F32, tag="qc")
                nc.sync.dma_start(qc[:], S[3:4, :])
                nc.vector.tensor_add(qa[:], qa[:], qb[:])
                nc.vector.tensor_add(qa[:], qa[:], qc[:])
                mh = spool.tile([1, nT], F32, tag="mh")
                nc.vector.tensor_scalar_mul(mh[:], qa[:], -0.5)
                # y rows to partition 0, then bf16 hi/lo
                ty = spool.tile([3, nT], F32, tag="ty")
                nc.sync.dma_start(ty[:], T[1:4, :])
                tyh = spool.tile([3, nT], BF16, tag="tyh")
                nc.vector.tensor_copy(tyh[:], ty[:])
                tyl = spool.tile([3, nT], BF16, tag="tyl")
                nc.vector.tensor_sub(tyl[:], ty[:], tyh[:])
                mhh = spool.tile([1, nT], BF16, tag="mhh")
                nc.vector.tensor_copy(mhh[:], mh[:])
                mhl = spool.tile([1, nT], BF16, tag="mhl")
                nc.vector.tensor_sub(mhl[:], mh[:], mhh[:])
                # place Y15 rows (rows 4, 9, 14 are constant, set at init)
                sl = slice(off, off + nT)
                nc.sync.dma_start(ynext[0:3, sl], tyh[:])
                nc.sync.dma_start(ynext[3:4, sl], mhh[:])
                nc.sync.dma_start(ynext[5:8, sl], tyl[:])
                nc.sync.dma_start(ynext[8:9, sl], mhl[:])
                nc.sync.dma_start(ynext[10:13, sl], tyh[:])
                nc.sync.dma_start(ynext[13:14, sl], mhh[:])
            pending.append(epilogue)

    while pending:
        pending.pop(0)()
    nc.sync.dma_start(yout[:], yout_t[:])
    ctx.close()


def build(num_iters=NUM_ITERS, groupw=GROUPW, o1bufs=2, ntiles=None):
    nc = bacc.Bacc("TRN2", target_bir_lowering=False, debug=False)
    aps = {
        "paug": nc.dram_tensor("paug", [10, N], BF16, kind="ExternalInput").ap(),
        "pts2": nc.dram_tensor("pts2", [128, 4 * NCHUNK], BF16,
                               kind="ExternalInput").ap(),
        "y015": nc.dram_tensor("y015", [10, Q], BF16, kind="ExternalInput").ap(),
        "yout": nc.dram_tensor("yout", [3, Q], F32, kind="ExternalOutput").ap(),
    }
    with tile.TileContext(nc) as tc:
        _emit(nc, tc, aps, num_iters, groupw, o1bufs, ntiles)
    nc.compile()
    return nc


def _hi_lo(a):
    """Split fp32 array into bf16 hi + bf16 lo (a ~ hi + lo)."""
    hi = a.astype(ml_dtypes.bfloat16)
    lo = (a - hi.astype(np.float32)).astype(ml_dtypes.bfloat16)
    return hi, lo


def _x5(p):
    """[5, n] rows [x0;x1;x2;1;-0.5||x||^2] for points p [n, 3] (lhsT side)."""
    n = p.shape[0]
    return np.concatenate(
        [p.T, np.ones((1, n), np.float32),
         -0.5 * (p * p).sum(1, dtype=np.float32)[None, :]], 0)


def _y5(p):
    """[5, n] rows [y0;y1;y2;-0.5||y||^2;1] for queries p [n, 3] (rhs side)."""
    n = p.shape[0]
    return np.concatenate(
        [p.T, -0.5 * (p * p).sum(1, dtype=np.float32)[None, :],
         np.ones((1, n), np.float32)], 0)


def make_in_maps(x):
    x = np.asarray(x, dtype=np.float32)
    ones18 = np.ones((1, 4), ml_dtypes.bfloat16)
    # amat [4, 20]: four [4, 5] column blocks applied to T_hi, T_lo, S_hi,
    # S_lo; output rows = [y0, y1, y2, -0.5||y||^2, 1]
    amat = np.zeros((4, 20), np.float32)
    for blk in (0, 5):          # T_hi / T_lo blocks
        for j in range(3):
            amat[1 + j, blk + j] = 1.0   # y_j
        amat[0, blk + 4] = 1.0           # ones row
    for blk in (10, 15):        # S_hi / S_lo blocks
        for j in range(3):
            amat[1 + j, blk + 3] = -0.5  # -0.5*sum y_j^2
    amat = amat.astype(ml_dtypes.bfloat16)
    in_maps = []
    for c in range(NCORES):
        b = c // CORES_PER_B
        pts = x[b].reshape(C, N).T.copy()          # [N, 3]
        q = pts[(c % CORES_PER_B) * Q:(c % CORES_PER_B + 1) * Q]  # [Q, 3]
        hiX, loX = _hi_lo(_x5(pts))
        paug = np.concatenate([hiX, loX], 0)       # [10, N] bf16
        hiY, loY = _hi_lo(_y5(q))
        y015 = np.concatenate([hiY, loY], 0)       # [10, Q] bf16
        a = pts.reshape(NCHUNK, CHUNK, C)
        cols = np.concatenate(
            [np.ones((NCHUNK, CHUNK, 1), np.float32), a], -1)  # [72,128,4]
        pts2 = np.ascontiguousarray(
            cols.transpose(1, 0, 2).reshape(CHUNK, 4 * NCHUNK)
        ).astype(ml_dtypes.bfloat16)
        in_maps.append({"paug": paug, "pts2": pts2, "y015": y015})
    return in_maps


def assemble(results):
    y = np.empty((B, C, N), np.float32)
    for c in range(NCORES):
        b = c // CORES_PER_B
        sl = slice((c % CORES_PER_B) * Q, (c % CORES_PER_B + 1) * Q)
        y[b, :, sl] = results[c]["yout"]
    return y.reshape(B, C, H, W)


class _CachedRunner:
    """run_bass_kernel_spmd's axon path (bass2jax.run_bass_via_pjrt) with the
    jitted SPMD executable cached across calls, so repeat invocations skip
    re-tracing/lowering. Math and execution mechanism are identical."""

    def __init__(self, nc, n_cores=NCORES):
        import jax
        from jax.sharding import Mesh, PartitionSpec
        from jax.experimental.shard_map import shard_map
        from concourse import bass2jax
        import concourse.mybir as mybir_

        bass2jax.install_neuronx_cc_hook()
        self.jax = jax
        in_names, out_names, out_avals, zero_outs = [], [], [], []
        partition_name = (nc.partition_id_tensor.name
                          if nc.partition_id_tensor else None)
        for alloc in nc.m.functions[0].allocations:
            if not isinstance(alloc, mybir_.MemoryLocationSet):
                continue
            name = alloc.memorylocations[0].name
            if alloc.kind == "ExternalInput":
                if name != partition_name:
                    in_names.append(name)
            elif alloc.kind == "ExternalOutput":
                out_names.append(name)
                shape = tuple(alloc.tensor_shape)
                dtype = mybir_.dt.np(alloc.dtype)
                out_avals.append(jax.core.ShapedArray(shape, dtype))
                zero_outs.append(np.zeros(shape, dtype))
        self.n_cores = n_cores
        self.in_names, self.out_names = in_names, out_names
        self.out_avals = out_avals
        self.zeros = [np.zeros((n_cores * z.shape[0], *z.shape[1:]), z.dtype)
                      for z in zero_outs]
        n_params, n_outs = len(in_names), len(out_avals)
        all_in = in_names + out_names
        if partition_name is not None:
            all_in = all_in + [partition_name]

        def _body(*args):
            operands = list(args)
            if partition_name is not None:
                operands.append(bass2jax.partition_id_tensor())
            return tuple(bass2jax._bass_exec_p.bind(
                *operands,
                out_avals=tuple(out_avals),
                in_names=tuple(all_in),
                out_names=tuple(out_names),
                lowering_input_output_aliases=(),
                sim_require_finite=True,
                sim_require_nnan=True,
                nc=nc,
            ))

        devices = jax.devices()[:n_cores]
        mesh = Mesh(np.asarray(devices), ("core",))
        self.fn = jax.jit(
            shard_map(_body, mesh=mesh,
                      in_specs=(PartitionSpec("core"),) * (n_params + n_outs),
                      out_specs=(PartitionSpec("core"),) * n_outs,
                      check_rep=False),
            donate_argnums=tuple(range(n_params, n_params + n_outs)),
            keep_unused=True,
        )

    def __call__(self, in_maps):
        per_core = [[np.asarray(m[n]) for n in self.in_names] for m in in_maps]
        concat_in = [
            np.concatenate([per_core[c][i] for c in range(self.n_cores)], 0)
            for i in range(len(self.in_names))]
        out = self.fn(*concat_in, *self.zeros)
        pulled = [np.asarray(o).reshape(self.n_cores, *av.shape)
                  for o, av in zip(out, self.out_avals)]
        return [{n: pulled[i][c] for i, n in enumerate(self.out_names)}
                for c in range(self.n_cores)]


_NC = None
_RUNNER = None


def kernel(x):
    global _NC, _RUNNER
    if _NC is None:
        _NC = build()
    in_maps = make_in_maps(x)
    if _RUNNER is None:
        try:
            _RUNNER = _CachedRunner(_NC)
        except Exception:
            _RUNNER = False
    if _RUNNER:
        try:
            return assemble(_RUNNER(in_maps))
        except Exception:
            pass
    res = run_bass_kernel_spmd(_NC, in_maps, core_ids=list(range(NCORES)))
    return assemble(res.results)


# revision 24
# speedup vs baseline: 1.8799x; 1.0708x over previous
"""Mean-shift filtering kernel for Trainium2, SPMD over 8 NeuronCores.

Algorithm (per core): flash-attention-style streaming over the N x N
Gaussian kernel matrix. Each core owns Q = N/4 query pixels of one batch
image (cores 0-3 -> batch 0, cores 4-7 -> batch 1) and the full point set
of that image.

Math: w[m,n] = exp(-||y_n - x_m||^2 / (2 bw^2))
            = exp(100 * (y_n.x_m - 0.5||y_n||^2 - 0.5||x_m||^2))
The inner term is ONE K=15 bf16 matmul via a compensated hi/lo split:
  out1 = hiX.hiY + hiX.loY + loX.hiY   (error ~1e-5 -> exp factor err ~1e-3)
with lhsT rows [hiX5; hiX5; loX5] and rhs rows [hiY5; loY5; hiY5], where
  X5 = [x0; x1; x2; 1; -0.5||x||^2],  Y5 = [y0; y1; y2; -0.5||y||^2; 1].
Then w = Exp(100 * out1) on ScalarE (PSUM -> SBUF bf16, grouped 3 PSUM banks
per activation to amortize the per-call ACT overhead), and a second bf16
matmul accumulates [den; num] over all point chunks:
  out2[4, n] += pts2[128, 4]^T @ w[128, n]  (pts2 rows = [1, x0, x1, x2]).
Epilogue (per n-tile): PE-free. r = 1/den (DVE reciprocal); the partition
broadcast of r, the cross-partition sum for ||y||^2, and all Y15 row
placements are single-row SBUF->SBUF DMAs (DMA is the only partition-crossing
engine; thin-K matmuls measured ~3-4x their streaming cost on HW, so PE is
kept exclusively for the two dense streams). DVE does the multiplies and the
bf16 hi/lo splits.

Scheduling: MM2 groups are emitted one group behind MM1s, and each tile's
final MM2 group + epilogue are deferred until after the next tile's second
group, so in PE program order the next tile's MM1s always precede
activation-blocked work -- keeps ScalarE gap-free across tile boundaries
(timeline-sim ACT occupancy ~90%).

MM1's stationary operand and rhs are K-padded from 15 to 128 rows with
zeros: matmul streaming cost is K-independent, and thin-K moving operands
measured ~600ns/matmul slower on HW (plus 128-row weights enable the fast
weight load path) -- worth ~215us/iteration.

PSUM: out1 2x[128,1536] (6 banks) + out2 2x[4,512] (2 banks) = 8 banks.
Engines: PE 2 cycles/kernel-element (the two dense matmul streams), ACT 1
element/cycle/lane, DVE + DMA queues take the whole epilogue.

HW wall-derived estimate ~135-180us/iteration, ~0.7-0.9ms for the full
5-iteration mean shift (pure-exp ScalarE roofline: 138us/iteration).
"""

import numpy as np
import ml_dtypes

import concourse.bass as bass
import concourse.tile as tile
from concourse import bacc, mybir
from concourse.bass_utils import run_bass_kernel_spmd

F32 = mybir.dt.float32
BF16 = mybir.dt.bfloat16

B, C, H, W = 2, 3, 96, 96
N = H * W            # 9216 points per image
NCORES = 8
CORES_PER_B = NCORES // B   # 4
Q = N // CORES_PER_B        # 2304 queries per core
NUM_ITERS = 5
BANDWIDTH = 0.1
SCALE = 1.0 / (BANDWIDTH * BANDWIDTH)  # 100.0 ; exp arg = SCALE * out1
CHUNK = 128
NCHUNK = N // CHUNK  # 72
# n-tiles within a core's Q queries: 512-wide (PSUM-bank aligned) + tail.
NTILES = [(0, 512), (512, 512), (1024, 512), (1536, 512), (2048, 256)]
GROUPW = 1536        # ACT group width = 3 PSUM banks of fp32


def _emit(nc, tc, aps, num_iters=NUM_ITERS, groupw=GROUPW, o1bufs=2, ntiles=None):
    paug, pts2, y015, yout = (
        aps["paug"], aps["pts2"], aps["y015"], aps["yout"])

    import contextlib
    ctx = contextlib.ExitStack()
    cpool = ctx.enter_context(tc.tile_pool(name="const", bufs=1))
    ypool = ctx.enter_context(tc.tile_pool(name="ybuf", bufs=2))
    wpool = ctx.enter_context(tc.tile_pool(name="w", bufs=4))
    spool = ctx.enter_context(tc.tile_pool(name="small", bufs=3))
    o1pool = ctx.enter_context(tc.tile_pool(name="out1", bufs=o1bufs, space="PSUM"))
    o2pool = ctx.enter_context(tc.tile_pool(name="out2", bufs=2, space="PSUM"))

    # resident inputs; paug DRAM is [10, N] = [hiX5; loX5], SBUF wants
    # [hiX5; hiX5; loX5] (pairs with Y15 = [hiY5; loY5; hiY5])
    # Load order: the first tile's dependencies (ya, paug) first so the
    # pipeline starts as early as possible; pts2/amat arrive under compute.
    ya = ypool.tile([128, Q], BF16, tag="ybuf")
    yb = ypool.tile([128, Q], BF16, tag="ybuf")
    nc.vector.memset(ya[:], 0.0)
    nc.vector.memset(yb[:], 0.0)
    nc.sync.dma_start(ya[0:5, :], y015[0:5, :])
    nc.sync.dma_start(ya[5:10, :], y015[5:10, :])
    nc.sync.dma_start(ya[10:15, :], y015[0:5, :])
    # yb's constant rows (ones row hi=1 / lo=0 and its duplicate)
    nc.sync.dma_start(yb[4:5, :], y015[4:5, :])
    nc.sync.dma_start(yb[9:10, :], y015[9:10, :])
    nc.sync.dma_start(yb[14:15, :], y015[4:5, :])
    # K padded 15 -> 128 with zero rows: streaming cost is K-independent and
    # 128-column/128-row weights enable the fast-weight-load path.
    paug_t = cpool.tile([128, N], BF16, tag="paug")
    nc.vector.memset(paug_t[:], 0.0)
    nc.sync.dma_start(paug_t[0:5, :], paug[0:5, :])
    nc.sync.dma_start(paug_t[5:10, :], paug[0:5, :])
    nc.sync.dma_start(paug_t[10:15, :], paug[5:10, :])
    pts2_t = cpool.tile([128, 4 * NCHUNK], BF16, tag="pts2")
    nc.sync.dma_start(pts2_t[:], pts2[:])
    yout_t = cpool.tile([3, Q], F32, tag="youtb")

    exp_fn = mybir.ActivationFunctionType.Exp

    pending = []
    for t in range(num_iters):
        ycur = ya if t % 2 == 0 else yb
        ynext = yb if t % 2 == 0 else ya
        last = t == num_iters - 1
        for (off, nT) in (ntiles or NTILES):
            gsz = groupw // nT  # chunks per ACT group
            out2 = o2pool.tile([4, nT], F32, tag="out2")

            def mm2(g, w, gsz=gsz, nT=nT, out2=out2):
                for j in range(gsz):
                    ch = g * gsz + j
                    nc.tensor.matmul(
                        out2[:],
                        pts2_t[:, ch * 4:(ch + 1) * 4],
                        w[:, j * nT:(j + 1) * nT],
                        start=(ch == 0), stop=(ch == NCHUNK - 1))

            # MM2s are emitted one group behind MM1s so that, in PE program
            # order, the next group's (and next tile's) MM1s precede MM2s
            # that block on the activation -- keeps ACT gap-free across
            # tile boundaries.
            mm2_prev = None
            for g in range(NCHUNK // gsz):
                out1 = o1pool.tile([128, groupw], F32, tag="out1")
                for j in range(gsz):
                    ch = g * gsz + j
                    nc.tensor.matmul(
                        out1[:, j * nT:(j + 1) * nT],
                        paug_t[:, ch * CHUNK:(ch + 1) * CHUNK],
                        ycur[:, off:off + nT],
                        start=True, stop=True)
                w = wpool.tile([128, groupw], BF16, tag="w")
                nc.scalar.activation(w[:], out1[:], exp_fn, scale=SCALE)
                if mm2_prev is not None:
                    mm2(*mm2_prev)
                mm2_prev = (g, w)
                if g == 1 and pending:
                    pending.pop(0)()
            mm2_last = mm2_prev
            # epilogue: divide, rebuild Y15 (or final output). All epilogue
            # matmuls are compensated hi/lo bf16 (1 cyc/col on PE instead of
            # fp32's 4). Emission is deferred (see below) so the scheduler
            # keeps feeding ACT with the next tile's groups first.
            def epilogue(out2=out2, off=off, nT=nT, last=last, ynext=ynext,
                         mm2=mm2, mm2_last=mm2_last):
                mm2(*mm2_last)  # deferred final MM2 group of this tile
                # PE-free epilogue: broadcasts and partition moves via SBUF
                # DMAs, arithmetic on DVE only (thin-K matmuls are slow on HW)
                r = spool.tile([1, nT], F32, tag="r")
                nc.vector.reciprocal(r[:], out2[0:1, :])
                o2c = spool.tile([4, nT], F32, tag="o2c")
                nc.vector.tensor_copy(o2c[:], out2[:])
                bcS = spool.tile([4, nT], F32, tag="bcS")
                for k in range(4):
                    nc.sync.dma_start(bcS[k:k + 1, :], r[:])
                T = spool.tile([4, nT], F32, tag="T")
                nc.vector.tensor_mul(T[:], o2c[:], bcS[:])  # [1, y0, y1, y2]
                if last:
                    nc.sync.dma_start(yout_t[:, off:off + nT], T[1:4, :])
                    return
                S = spool.tile([4, nT], F32, tag="S")
                nc.vector.tensor_mul(S[:], T[:], T[:])
                # ysq = sum of S rows 1..3 via row DMAs to partition 0
                qa = spool.tile([1, nT], F32, tag="qa")
                nc.sync.dma_start(qa[:], S[1:2, :])
                qb = spool.tile([1, nT], F32, tag="qb")
                nc.sync.dma_start(qb[:], S[2:3, :])
                qc = spool.tile([1, nT], F32, tag="qc")
                nc.sync.dma_start(qc[:], S[3:4, :])
                nc.vector.tensor_add(qa[:], qa[:], qb[:])
                nc.vector.tensor_add(qa[:], qa[:], qc[:])
                mh = spool.tile([1, nT], F32, tag="mh")
                nc.vector.tensor_scalar_mul(mh[:], qa[:], -0.5)
                # y rows to partition 0, then bf16 hi/lo
                ty = spool.tile([3, nT], F32, tag="ty")
                nc.sync.dma_start(ty[:], T[1:4, :])
                tyh = spool.tile([3, nT], BF16, tag="tyh")
                nc.vector.tensor_copy(tyh[:], ty[:])
                tyl = spool.tile([3, nT], BF16, tag="tyl")
                nc.vector.tensor_sub(tyl[:], ty[:], tyh[:])
                mhh = spool.tile([1, nT], BF16, tag="mhh")
                nc.vector.tensor_copy(mhh[:], mh[:])
                mhl = spool.tile([1, nT], BF16, tag="mhl")
                nc.vector.tensor_sub(mhl[:], mh[:], mhh[:])
                # place Y15 rows (rows 4, 9, 14 are constant, set at init)
                sl = slice(off, off + nT)
                nc.sync.dma_start(ynext[0:3, sl], tyh[:])
                nc.sync.dma_start(ynext[3:4, sl], mhh[:])
                nc.sync.dma_start(ynext[5:8, sl], tyl[:])
                nc.sync.dma_start(ynext[8:9, sl], mhl[:])
                nc.sync.dma_start(ynext[10:13, sl], tyh[:])
                nc.sync.dma_start(ynext[13:14, sl], mhh[:])
            pending.append(epilogue)

    while pending:
        pending.pop(0)()
    nc.sync.dma_start(yout[:], yout_t[:])
    ctx.close()


def build(num_iters=NUM_ITERS, groupw=GROUPW, o1bufs=2, ntiles=None):
    nc = bacc.Bacc("TRN2", target_bir_lowering=False, debug=False)
    aps = {
        "paug": nc.dram_tensor("paug", [10, N], BF16, kind="ExternalInput").ap(),
        "pts2": nc.dram_tensor("pts2", [128, 4 * NCHUNK], BF16,
                               kind="ExternalInput").ap(),
        "y015": nc.dram_tensor("y015", [10, Q], BF16, kind="ExternalInput").ap(),
        "yout": nc.dram_tensor("yout", [3, Q], F32, kind="ExternalOutput").ap(),
    }
    with tile.TileContext(nc) as tc:
        _emit(nc, tc, aps, num_iters, groupw, o1bufs, ntiles)
    nc.compile()
    return nc


def _hi_lo(a):
    """Split fp32 array into bf16 hi + bf16 lo (a ~ hi + lo)."""
    hi = a.astype(ml_dtypes.bfloat16)
    lo = (a - hi.astype(np.float32)).astype(ml_dtypes.bfloat16)
    return hi, lo


def _x5(p):
    """[5, n] rows [x0;x1;x2;1;-0.5||x||^2] for points p [n, 3] (lhsT side)."""
    n = p.shape[0]
    return np.concatenate(
        [p.T, np.ones((1, n), np.float32),
         -0.5 * (p * p).sum(1, dtype=np.float32)[None, :]], 0)


def _y5(p):
    """[5, n] rows [y0;y1;y2;-0.5||y||^2;1] for queries p [n, 3] (rhs side)."""
    n = p.shape[0]
    return np.concatenate(
        [p.T, -0.5 * (p * p).sum(1, dtype=np.float32)[None, :],
         np.ones((1, n), np.float32)], 0)


def make_in_maps(x):
    x = np.asarray(x, dtype=np.float32)
    ones18 = np.ones((1, 4), ml_dtypes.bfloat16)
    # amat [4, 20]: four [4, 5] column blocks applied to T_hi, T_lo, S_hi,
    # S_lo; output rows = [y0, y1, y2, -0.5||y||^2, 1]
    amat = np.zeros((4, 20), np.float32)
    for blk in (0, 5):          # T_hi / T_lo blocks
        for j in range(3):
            amat[1 + j, blk + j] = 1.0   # y_j
        amat[0, blk + 4] = 1.0           # ones row
    for blk in (10, 15):        # S_hi / S_lo blocks
        for j in range(3):
            amat[1 + j, blk + 3] = -0.5  # -0.5*sum y_j^2
    amat = amat.astype(ml_dtypes.bfloat16)
    in_maps = []
    for c in range(NCORES):
        b = c // CORES_PER_B
        pts = x[b].reshape(C, N).T.copy()          # [N, 3]
        q = pts[(c % CORES_PER_B) * Q:(c % CORES_PER_B + 1) * Q]  # [Q, 3]
        hiX, loX = _hi_lo(_x5(pts))
        paug = np.concatenate([hiX, loX], 0)       # [10, N] bf16
        hiY, loY = _hi_lo(_y5(q))
        y015 = np.concatenate([hiY, loY], 0)       # [10, Q] bf16
        a = pts.reshape(NCHUNK, CHUNK, C)
        cols = np.concatenate(
            [np.ones((NCHUNK, CHUNK, 1), np.float32), a], -1)  # [72,128,4]
        pts2 = np.ascontiguousarray(
            cols.transpose(1, 0, 2).reshape(CHUNK, 4 * NCHUNK)
        ).astype(ml_dtypes.bfloat16)
        in_maps.append({"paug": paug, "pts2": pts2, "y015": y015})
    return in_maps


def assemble(results):
    y = np.empty((B, C, N), np.float32)
    for c in range(NCORES):
        b = c // CORES_PER_B
        sl = slice((c % CORES_PER_B) * Q, (c % CORES_PER_B + 1) * Q)
        y[b, :, sl] = results[c]["yout"]
    return y.reshape(B, C, H, W)


class _CachedRunner:
    """run_bass_kernel_spmd's axon path (bass2jax.run_bass_via_pjrt) with the
    jitted SPMD executable cached across calls, so repeat invocations skip
    re-tracing/lowering. Math and execution mechanism are identical."""

    def __init__(self, nc, n_cores=NCORES):
        import jax
        from jax.sharding import Mesh, PartitionSpec
        from jax.experimental.shard_map import shard_map
        from concourse import bass2jax
        import concourse.mybir as mybir_

        bass2jax.install_neuronx_cc_hook()
        self.jax = jax
        in_names, out_names, out_avals, zero_outs = [], [], [], []
        partition_name = (nc.partition_id_tensor.name
                          if nc.partition_id_tensor else None)
        for alloc in nc.m.functions[0].allocations:
            if not isinstance(alloc, mybir_.MemoryLocationSet):
                continue
            name = alloc.memorylocations[0].name
            if alloc.kind == "ExternalInput":
                if name != partition_name:
                    in_names.append(name)
            elif alloc.kind == "ExternalOutput":
                out_names.append(name)
                shape = tuple(alloc.tensor_shape)
                dtype = mybir_.dt.np(alloc.dtype)
                out_avals.append(jax.core.ShapedArray(shape, dtype))
                zero_outs.append(np.zeros(shape, dtype))
        self.n_cores = n_cores
        self.in_names, self.out_names = in_names, out_names
        self.out_avals = out_avals
        self.zeros = [np.zeros((n_cores * z.shape[0], *z.shape[1:]), z.dtype)
                      for z in zero_outs]
        n_params, n_outs = len(in_names), len(out_avals)
        all_in = in_names + out_names
        if partition_name is not None:
            all_in = all_in + [partition_name]

        def _body(*args):
            operands = list(args)
            if partition_name is not None:
                operands.append(bass2jax.partition_id_tensor())
            return tuple(bass2jax._bass_exec_p.bind(
                *operands,
                out_avals=tuple(out_avals),
                in_names=tuple(all_in),
                out_names=tuple(out_names),
                lowering_input_output_aliases=(),
                sim_require_finite=True,
                sim_require_nnan=True,
                nc=nc,
            ))

        devices = jax.devices()[:n_cores]
        mesh = Mesh(np.asarray(devices), ("core",))
        self.fn = jax.jit(
            shard_map(_body, mesh=mesh,
                      in_specs=(PartitionSpec("core"),) * (n_params + n_outs),
                      out_specs=(PartitionSpec("core"),) * n_outs,
                      check_rep=False),
            donate_argnums=tuple(range(n_params, n_params + n_outs)),
            keep_unused=True,
        )

    def __call__(self, in_maps):
        per_core = [[np.asarray(m[n]) for n in self.in_names] for m in in_maps]
        concat_in = [
            np.concatenate([per_core[c][i] for c in range(self.n_cores)], 0)
            for i in range(len(self.in_names))]
        out = self.fn(*concat_in, *self.zeros)
        pulled = [np.asarray(o).reshape(self.n_cores, *av.shape)
                  for o, av in zip(out, self.out_avals)]
        return [{n: pulled[i][c] for i, n in enumerate(self.out_names)}
                for c in range(self.n_cores)]


_NC = None
_RUNNER = None


def kernel(x):
    global _NC, _RUNNER
    if _NC is None:
        _NC = build()
    in_maps = make_in_maps(x)
    if _RUNNER is None:
        try:
            _RUNNER = _CachedRunner(_NC)
        except Exception:
            _RUNNER = False
    if _RUNNER:
        try:
            return assemble(_RUNNER(in_maps))
        except Exception:
            pass
    res = run_bass_kernel_spmd(_NC, in_maps, core_ids=list(range(NCORES)))
    return assemble(res.results)
